# revision 13
# baseline (speedup 1.0000x reference)
"""Trainium2 Bass kernel for single-token multi-head attention with KV cache
(B=16, S=1, D=2048, H=16, Dh=128, MAX_SEQ=4096), tensor-parallel over heads
across 8 NeuronCores (2 heads per core).

Per core:
  - q/k/v projections for the core's 2 heads (column-sliced Wq/Wk/Wv),
  - RoPE on q/k, KV-cache update at position `start_position`,
  - attention over the cached prefix (the memory-bound part: each core
    streams its 2-head slice of the K and V caches, ~134 MB),
  - partial output projection with the row-slice of Wo.
The host sums the 8 partial [B, D] outputs (tensor-parallel unshard).

Layouts are staged host-side so every large DMA reads contiguous 16 KB
per-partition lines:
  kt: [pair, Dh, T]   (K transposed: scores matmul keeps K chunks stationary)
  vt: [pair, q, c*128+j] with t = c*128+q  (V partitioned by t%128)
"""

import math
import sys
import types

sys.path.insert(0, "/opt/trn_rl_repo")

import numpy as np
import ml_dtypes

import concourse.bass as bass
import concourse.mybir as mybir
import concourse.tile as tile
from concourse.bass_utils import run_bass_kernel_spmd
from concourse.masks import make_identity

B, D, H, DH = 16, 2048, 16, 128
NCORES = 8
HLOC = H // NCORES  # heads per core
NPAIR = HLOC * B  # (head, batch) pairs per core
FP32 = mybir.dt.float32
BF16 = mybir.dt.bfloat16
SCALE = 1.0 / math.sqrt(DH)

LAST_RESULT = None  # BassKernelResults of the most recent run (for test harness)


def _split_multi_waits(nc):
    """walrus in this container accepts at most ONE sync wait per instruction
    (setupSyncWait: "Too many sync wait commands"). Tile's scheduler attaches
    several. Hoist all but the last wait of each instruction onto wait-only
    EventSemaphore instructions inserted right before it on the same engine —
    per-engine program order makes this semantically identical."""
    for f in nc.m.functions:
        for blk in f.blocks:
            insts = blk.instructions
            if not any(
                i.sync_info is not None and len(i.sync_info.on_wait) > 1
                for i in insts
            ):
                continue
            new = []
            for inst in insts:
                si = inst.sync_info
                if si is not None and len(si.on_wait) > 1:
                    waits = list(si.on_wait)
                    for j, w in enumerate(waits[:-1]):
                        es = mybir.InstEventSemaphore(
                            name=f"{inst.name}_hw{j}",
                            ins=[],
                            outs=[],
                            engine=inst.engine,
                        )
                        es.sync_info = mybir.SyncInfo(on_wait=[w], on_update=[])
                        new.append(es)
                    inst.sync_info = mybir.SyncInfo(
                        on_wait=[waits[-1]], on_update=list(si.on_update)
                    )
                new.append(inst)
            blk.instructions = new


def _build_program(start):
    """Bass program for one core (SPMD: all 8 cores run the same program on
    different data). `start` is the KV-cache write position; attention spans
    t in [0, start]."""
    nch = start // 128 + 1  # T-chunks of 128, padded
    Tp = nch * 128
    r = start % 128  # t=start lives at partition r of chunk nch-1
    c_last = nch - 1

    nc = bass.Bass(
        "TRN2", target_bir_lowering=False, debug=False, num_devices=NCORES
    )

    xT3 = nc.dram_tensor("xT3", [D // 128, 128, B], BF16, kind="ExternalInput")
    wq3 = nc.dram_tensor("wq3", [D // 128, 128, HLOC * DH], BF16, kind="ExternalInput")
    wk3 = nc.dram_tensor("wk3", [D // 128, 128, HLOC * DH], BF16, kind="ExternalInput")
    wv3 = nc.dram_tensor("wv3", [D // 128, 128, HLOC * DH], BF16, kind="ExternalInput")
    wo3 = nc.dram_tensor("wo3", [HLOC, 128, D], BF16, kind="ExternalInput")
    cosr = nc.dram_tensor("cosr", [1, HLOC * DH], FP32, kind="ExternalInput")
    sinr = nc.dram_tensor("sinr", [1, HLOC * DH], FP32, kind="ExternalInput")
    kv3 = nc.dram_tensor("kv3", [NPAIR, 2, 128, Tp], BF16, kind="ExternalInput")
    outp = nc.dram_tensor("outp", [B, D], FP32, kind="ExternalOutput")

    W = HLOC * DH  # 256: q/k/v row width for this core's heads
    Exp = mybir.ActivationFunctionType.Exp
    mult = mybir.AluOpType.mult
    add = mybir.AluOpType.add

    with tile.TileContext(nc) as tc:
        with (
            tc.tile_pool(name="consts", bufs=1) as consts,
            tc.tile_pool(name="sb", bufs=1) as sb,
            tc.tile_pool(name="wts", bufs=1) as wts,
            tc.tile_pool(name="kpool", bufs=4) as kpool,
            tc.tile_pool(name="etp", bufs=3) as etp,
        ):
            # ---- constants ----
            identity = consts.tile([128, 128], FP32, tag="identity")
            make_identity(nc, identity[:])
            ones_colf = consts.tile([128, 1], FP32, tag="ones_colf")
            nc.vector.memset(ones_colf[:], 1.0)
            ones_row = consts.tile([1, 128], FP32, tag="ones_row")
            nc.vector.memset(ones_row[:], 1.0)
            cos_sb = consts.tile([B, W], FP32, tag="cos")
            sin_sb = consts.tile([B, W], FP32, tag="sin")
            nc.gpsimd.dma_start(cos_sb[:], cosr.ap().to_broadcast((B, W)))
            nc.gpsimd.dma_start(sin_sb[:], sinr.ap().to_broadcast((B, W)))

            # ---- phase A: projections + RoPE + transposes ----
            xs = sb.tile([128, D // 128, B], BF16, tag="xs")
            nc.gpsimd.dma_start(xs[:], xT3.ap().rearrange("c p b -> p c b"))
            wq_sb = wts.tile([128, D // 128, W], BF16, tag="wq")
            wk_sb = wts.tile([128, D // 128, W], BF16, tag="wk")
            wv_sb = wts.tile([128, D // 128, W], BF16, tag="wv")
            wo_sb = wts.tile([128, HLOC, D], BF16, tag="wo")
            nc.sync.dma_start(wq_sb[:], wq3.ap().rearrange("c p n -> p c n"))
            nc.sync.dma_start(wk_sb[:], wk3.ap().rearrange("c p n -> p c n"))
            nc.sync.dma_start(wv_sb[:], wv3.ap().rearrange("c p n -> p c n"))

            qT_sb = sb.tile([128, NPAIR], BF16, tag="qT")
            kT_sb = sb.tile([128, NPAIR], BF16, tag="kT")
            vrows = sb.tile([B, W], BF16, tag="vrows")

            with tc.tile_pool(name="psA", bufs=2, space="PSUM") as psA:
                rots = {}
                for wname, w_sb in (("q", wq_sb), ("k", wk_sb), ("v", wv_sb)):
                    prj = psA.tile([B, W], FP32, tag="prj")
                    for ci in range(D // 128):
                        nc.tensor.matmul(
                            prj[:],
                            xs[:, ci, :],
                            w_sb[:, ci, :],
                            start=(ci == 0),
                            stop=(ci == D // 128 - 1),
                        )
                    if wname == "v":
                        nc.vector.tensor_copy(vrows[:], prj[:])
                        continue
                    # RoPE in row layout: rot = prj*cos + swap(prj)*sin_signed
                    sw = sb.tile([B, W], FP32, tag="ropesw")
                    p3 = prj[:].rearrange("b (i two) -> b i two", two=2)
                    s3 = sw[:].rearrange("b (i two) -> b i two", two=2)
                    nc.vector.tensor_copy(s3[:, :, 0], p3[:, :, 1])
                    nc.vector.tensor_copy(s3[:, :, 1], p3[:, :, 0])
                    t1 = sb.tile([B, W], FP32, tag="ropet1")
                    t2 = sb.tile([B, W], FP32, tag="ropet2")
                    nc.vector.tensor_tensor(t1[:], prj[:], cos_sb[:], op=mult)
                    nc.vector.tensor_tensor(t2[:], sw[:], sin_sb[:], op=mult)
                    rot = sb.tile([B, W], FP32, tag=f"rot_{wname}")
                    nc.vector.tensor_tensor(rot[:], t1[:], t2[:], op=add)
                    rots[wname] = rot

                for h in range(HLOC):
                    for rot, dst in ((rots["q"], qT_sb), (rots["k"], kT_sb)):
                        tps = psA.tile([128, B], FP32, tag="tps")
                        nc.tensor.transpose(
                            tps[:],
                            rot[:, h * DH : (h + 1) * DH],
                            identity[:B, :B],
                        )
                        nc.vector.tensor_copy(
                            dst[:, h * B : (h + 1) * B], tps[:]
                        )

            # ---- phase B: attention over the cached prefix ----
            # Software-pipelined over pairs: pair p's V-matmuls are emitted
            # after pair p+1's score-matmuls so the PE never waits on the
            # exp round trip; K and V arrive in one merged 2MB DMA per pair.
            # per-pair softmax denominators accumulate for free via the exp's
            # accum_out; zero-padded tail columns each contribute exactly
            # exp(0) = 1, corrected with a compile-time constant below.
            accs = sb.tile([128, NPAIR], FP32, tag="accs")
            out_sb = sb.tile([B, D], FP32, tag="outsb")
            out_fin = sb.tile([B, D], FP32, tag="outfin")
            attn_sbs = []
            with (
                tc.tile_pool(name="psB", bufs=2, space="PSUM") as psB,
                tc.tile_pool(name="psacc", bufs=2, space="PSUM") as psacc,
                tc.tile_pool(name="psC", bufs=2, space="PSUM") as psC,
            ):
                attn_pss = []

                def emit_normalize(h):
                    # attn_sb = attn_ps * (1/sum); K=1 ones-matmul broadcasts
                    # the per-batch scalars across partitions
                    sums = psB.tile([1, B], FP32, tag="misc")
                    nc.tensor.matmul(
                        sums[:],
                        ones_colf[:],
                        accs[:, h * B : (h + 1) * B],
                        start=True,
                        stop=True,
                    )
                    stot_h = sb.tile([1, B], FP32, tag=f"stot{h}")
                    if r < 127:
                        nc.vector.tensor_scalar_add(
                            stot_h[:], sums[:], float(-(127 - r))
                        )
                    else:
                        nc.vector.tensor_copy(stot_h[:], sums[:])
                    inv_sb = sb.tile([1, B], FP32, tag=f"inv{h}")
                    nc.vector.reciprocal(inv_sb[:], stot_h[:])
                    binv = psB.tile([128, B], FP32, tag="misc")
                    nc.tensor.matmul(
                        binv[:], ones_row[:], inv_sb[:], start=True, stop=True
                    )
                    binv_sb = sb.tile([128, B], FP32, tag=f"binv{h}")
                    nc.vector.tensor_copy(binv_sb[:], binv[:])
                    attn_sb = sb.tile([128, B], BF16, tag=f"attnsb{h}")
                    nc.vector.tensor_tensor(
                        attn_sb[:], attn_pss[h][:], binv_sb[:], op=mult
                    )
                    attn_sbs.append(attn_sb)
                    # this head's slice of the output projection; head 0's
                    # runs mid-stream in PE slack, head 1's in the tail
                    for nt in range(D // 512):
                        ops = psC.tile([B, 512], FP32, tag="ops")
                        nc.tensor.matmul(
                            ops[:],
                            attn_sb[:],
                            wo_sb[:, h, nt * 512 : (nt + 1) * 512],
                            start=True,
                            stop=True,
                        )
                        dst = out_sb if h == 0 else out_fin
                        if h == 0:
                            nc.vector.tensor_copy(
                                dst[:, nt * 512 : (nt + 1) * 512], ops[:]
                            )
                        else:
                            nc.vector.tensor_tensor(
                                dst[:, nt * 512 : (nt + 1) * 512],
                                ops[:],
                                out_sb[:, nt * 512 : (nt + 1) * 512],
                                op=add,
                            )

                def emit_v(h, b, et, vt_v):
                    for ci in range(nch):
                        nc.tensor.matmul(
                            attn_pss[h][:, b : b + 1],
                            vt_v[:, ci * 128 : (ci + 1) * 128],
                            et[:, ci : ci + 1],
                            start=(ci == 0),
                            stop=(ci == nch - 1),
                        )
                    if b == B - 1:
                        emit_normalize(h)

                pending = None
                for h in range(HLOC):
                    attn_ps = psacc.tile([128, B], FP32, tag="attn")
                    attn_pss.append(attn_ps)
                    for b in range(B):
                        pcol = h * B + b
                        if pcol == 2:
                            # wo isn't needed until the first normalize; keep
                            # the startup window free for the q/k/v weights
                            nc.sync.dma_start(
                                wo_sb[:], wo3.ap().rearrange("h p n -> p h n")
                            )
                        kv_t = kpool.tile([128, 2, Tp], BF16, tag="kv")
                        nc.sync.dma_start(kv_t[:, 0, :], kv3.ap()[pcol, 0])
                        nc.sync.dma_start(kv_t[:, 1, :], kv3.ap()[pcol, 1])
                        kt_v = kv_t[:, 0, :]
                        vt_v = kv_t[:, 1, :]
                        # insert this step's (RoPE'd) k / v at t=start
                        nc.vector.tensor_copy(
                            kt_v[:, start : start + 1], kT_sb[:, pcol : pcol + 1]
                        )
                        nc.gpsimd.dma_start(
                            vt_v[r : r + 1, c_last * 128 : (c_last + 1) * 128],
                            vrows[b : b + 1, h * DH : (h + 1) * DH],
                        )
                        sc = psB.tile([128, nch], FP32, tag="sc")
                        for ci in range(nch):
                            nc.tensor.matmul(
                                sc[:, ci : ci + 1],
                                kt_v[:, ci * 128 : (ci + 1) * 128],
                                qT_sb[:, pcol : pcol + 1],
                                start=True,
                                stop=True,
                            )
                        if pending is not None:
                            emit_v(*pending)
                        et = etp.tile([128, nch], BF16, tag="et")
                        nc.scalar.activation(
                            et[:],
                            sc[:],
                            Exp,
                            scale=SCALE,
                            accum_out=accs[:, pcol : pcol + 1],
                        )
                        pending = (h, b, et, vt_v)
                emit_v(*pending)
            nc.sync.dma_start(outp.ap(), out_fin[:])

    _split_multi_waits(nc)
    return nc


_programs = {}


def _get_program(start):
    if start not in _programs:
        _programs[start] = _build_program(start)
    return _programs[start]


def _stage_inputs(inputs, key_cache, value_cache, freqs_cos, freqs_sin, Wq, Wk, Wv, Wo, start):
    nch = start // 128 + 1
    Tp = nch * 128
    r = start % 128

    f32 = np.float32
    bf16 = ml_dtypes.bfloat16
    x = np.asarray(inputs, f32).reshape(B, D)
    xT3 = np.ascontiguousarray(x.T, dtype=bf16).reshape(D // 128, 128, B)

    kc = np.asarray(key_cache, f32)[:, :Tp]  # [B, Tp, H, DH]
    vc = np.asarray(value_cache, f32)[:, :Tp]
    # One merged bf16 array per (head, batch) pair: [..., 0] = K^T [DH, Tp],
    # [..., 1] = V tiled [q, c*128+j] with t = c*128+q. bf16 halves the DMA
    # traffic; scores/attention still accumulate in fp32 on the PE.
    kv_all = np.empty((H, B, 2, 128, Tp), dtype=ml_dtypes.bfloat16)
    kv_all[:, :, 0] = kc.transpose(2, 0, 3, 1)
    kv_all[:, :, 1] = (
        vc.reshape(B, nch, 128, H, DH).transpose(3, 0, 2, 1, 4).reshape(H, B, 128, Tp)
    )
    if start + 1 < Tp:
        kv_all[:, :, 0, :, start + 1 :] = 0.0
        kv_all[:, :, 1, r + 1 :, (nch - 1) * 128 :] = 0.0

    fc = np.asarray(freqs_cos, f32).reshape(-1)[: DH // 2]
    fs = np.asarray(freqs_sin, f32).reshape(-1)[: DH // 2]
    cos128 = np.repeat(fc, 2)
    sin128 = np.repeat(fs, 2) * np.tile(np.array([-1.0, 1.0], f32), DH // 2)
    cos_row = np.ascontiguousarray(np.tile(cos128, HLOC)[None, :], dtype=f32)
    sin_row = np.ascontiguousarray(np.tile(sin128, HLOC)[None, :], dtype=f32)

    Wq = np.asarray(Wq, f32)
    Wk = np.asarray(Wk, f32)
    Wv = np.asarray(Wv, f32)
    Wo = np.asarray(Wo, f32)

    in_maps = []
    for c in range(NCORES):
        hs = slice(HLOC * c, HLOC * (c + 1))
        cols = slice(HLOC * c * DH, HLOC * (c + 1) * DH)
        in_maps.append(
            {
                "xT3": xT3,
                "wq3": np.ascontiguousarray(Wq[:, cols], dtype=bf16).reshape(D // 128, 128, HLOC * DH),
                "wk3": np.ascontiguousarray(Wk[:, cols], dtype=bf16).reshape(D // 128, 128, HLOC * DH),
                "wv3": np.ascontiguousarray(Wv[:, cols], dtype=bf16).reshape(D // 128, 128, HLOC * DH),
                "wo3": np.ascontiguousarray(Wo[cols, :], dtype=bf16).reshape(HLOC, 128, D),
                "cosr": cos_row,
                "sinr": sin_row,
                "kv3": kv_all[hs].reshape(NPAIR, 2, 128, Tp),
            }
        )
    return in_maps


def kernel(
    inputs,
    key_cache,
    value_cache,
    freqs_cos,
    freqs_sin,
    Wq,
    Wk,
    Wv,
    Wo,
    start_position,
    _trace=False,
    _tmpdir=None,
):
    global LAST_RESULT
    start = int(start_position)
    nc = _get_program(start)
    in_maps = _stage_inputs(
        inputs, key_cache, value_cache, freqs_cos, freqs_sin, Wq, Wk, Wv, Wo, start
    )
    res = run_bass_kernel_spmd(
        nc,
        in_maps,
        core_ids=list(range(NCORES)),
        trace=_trace,
        tmpdir=_tmpdir,
    )
    LAST_RESULT = res
    out = np.zeros((B, D), np.float32)
    for c in range(NCORES):
        out += res.results[c]["outp"]
    return out.reshape(B, 1, D)


# revision 14
# speedup vs baseline: 1.0709x; 1.0709x over previous
"""Trainium2 Bass kernel for single-token multi-head attention with KV cache
(B=16, S=1, D=2048, H=16, Dh=128, MAX_SEQ=4096), tensor-parallel over heads
across 8 NeuronCores (2 heads per core).

Per core:
  - q/k/v projections for the core's 2 heads (column-sliced Wq/Wk/Wv),
  - RoPE on q/k, KV-cache update at position `start_position`,
  - attention over the cached prefix (the memory-bound part: each core
    streams its 2-head slice of the K and V caches, 67 MB as bf16),
  - partial output projection with the row-slice of Wo.
The host sums the 8 partial [B, D] outputs (tensor-parallel unshard).

The K/V cache slices are staged host-side in bf16 (halves HBM traffic; all
matmuls still accumulate in fp32, softmax statistics are fp32) with layouts
chosen so every large DMA reads contiguous per-partition lines:
  kv[pair][0] = K^T  [Dh, Tp]               (scores keep K chunks stationary)
  kv[pair][1] = V as [q, c*128+j], t=c*128+q (V partitioned by t mod 128)
The per-pair GEMV attention runs as 128x128-stationary matmuls with N=1
moving vectors; exp runs on the scalar engine with accumulated row sums;
softmax denominators and normalization use ones-matmul partition reductions.
The pair loop is software-pipelined (pair p's V-matmuls are emitted after
pair p+1's score-matmuls) so the PE never stalls on the exp round trip.
"""

import math
import sys

sys.path.insert(0, "/opt/trn_rl_repo")

import numpy as np
import ml_dtypes

import concourse.bass as bass
import concourse.mybir as mybir
import concourse.tile as tile
from concourse.bass_utils import run_bass_kernel_spmd
from concourse.masks import make_identity

B, D, H, DH = 16, 2048, 16, 128
NCORES = 8
HLOC = H // NCORES  # heads per core
NPAIR = HLOC * B  # (head, batch) pairs per core
FP32 = mybir.dt.float32
BF16 = mybir.dt.bfloat16
SCALE = 1.0 / math.sqrt(DH)

LAST_RESULT = None  # BassKernelResults of the most recent run (for test harness)


def _split_multi_waits(nc):
    """walrus in this container accepts at most ONE sync wait per instruction
    (setupSyncWait: "Too many sync wait commands"). Tile's scheduler attaches
    several. Hoist all but the last wait of each instruction onto wait-only
    EventSemaphore instructions inserted right before it on the same engine —
    per-engine program order makes this semantically identical."""
    for f in nc.m.functions:
        for blk in f.blocks:
            insts = blk.instructions
            if not any(
                i.sync_info is not None and len(i.sync_info.on_wait) > 1
                for i in insts
            ):
                continue
            new = []
            for inst in insts:
                si = inst.sync_info
                if si is not None and len(si.on_wait) > 1:
                    waits = list(si.on_wait)
                    for j, w in enumerate(waits[:-1]):
                        es = mybir.InstEventSemaphore(
                            name=f"{inst.name}_hw{j}",
                            ins=[],
                            outs=[],
                            engine=inst.engine,
                        )
                        es.sync_info = mybir.SyncInfo(on_wait=[w], on_update=[])
                        new.append(es)
                    inst.sync_info = mybir.SyncInfo(
                        on_wait=[waits[-1]], on_update=list(si.on_update)
                    )
                new.append(inst)
            blk.instructions = new


def _build_program(start):
    """Bass program for one core (SPMD: all 8 cores run the same program on
    different data). `start` is the KV-cache write position; attention spans
    t in [0, start]."""
    nch = start // 128 + 1  # T-chunks of 128, padded
    Tp = nch * 128
    r = start % 128  # t=start lives at partition r of chunk nch-1
    c_last = nch - 1

    nc = bass.Bass(
        "TRN2", target_bir_lowering=False, debug=False, num_devices=NCORES
    )

    xT3 = nc.dram_tensor("xT3", [D // 128, 128, B], BF16, kind="ExternalInput")
    wq3 = nc.dram_tensor("wq3", [D // 128, 128, HLOC * DH], BF16, kind="ExternalInput")
    wk3 = nc.dram_tensor("wk3", [D // 128, 128, HLOC * DH], BF16, kind="ExternalInput")
    wv3 = nc.dram_tensor("wv3", [D // 128, 128, HLOC * DH], BF16, kind="ExternalInput")
    wo3 = nc.dram_tensor("wo3", [HLOC, 128, D], BF16, kind="ExternalInput")
    cosr = nc.dram_tensor("cosr", [1, HLOC * DH], FP32, kind="ExternalInput")
    sinr = nc.dram_tensor("sinr", [1, HLOC * DH], FP32, kind="ExternalInput")
    kv3 = nc.dram_tensor("kv3", [NPAIR, 2, 128, Tp], BF16, kind="ExternalInput")
    outp = nc.dram_tensor("outp", [B, D], FP32, kind="ExternalOutput")

    W = HLOC * DH  # 256: q/k/v row width for this core's heads
    Exp = mybir.ActivationFunctionType.Exp
    mult = mybir.AluOpType.mult
    add = mybir.AluOpType.add

    with tile.TileContext(nc) as tc:
        with (
            tc.tile_pool(name="consts", bufs=1) as consts,
            tc.tile_pool(name="sb", bufs=1) as sb,
            tc.tile_pool(name="wts", bufs=1) as wts,
            tc.tile_pool(name="kpool", bufs=4) as kpool,
            tc.tile_pool(name="etp", bufs=3) as etp,
        ):
            # ---- constants ----
            identity = consts.tile([128, 128], FP32, tag="identity")
            make_identity(nc, identity[:])
            ones_colf = consts.tile([128, 1], FP32, tag="ones_colf")
            nc.vector.memset(ones_colf[:], 1.0)
            ones_row = consts.tile([1, 128], FP32, tag="ones_row")
            nc.vector.memset(ones_row[:], 1.0)
            cos_sb = consts.tile([B, W], FP32, tag="cos")
            sin_sb = consts.tile([B, W], FP32, tag="sin")
            nc.gpsimd.dma_start(cos_sb[:], cosr.ap().to_broadcast((B, W)))
            nc.gpsimd.dma_start(sin_sb[:], sinr.ap().to_broadcast((B, W)))

            # ---- phase A: projections + RoPE + transposes ----
            xs = sb.tile([128, D // 128, B], BF16, tag="xs")
            nc.gpsimd.dma_start(xs[:], xT3.ap().rearrange("c p b -> p c b"))
            wq_sb = wts.tile([128, D // 128, W], BF16, tag="wq")
            wk_sb = wts.tile([128, D // 128, W], BF16, tag="wk")
            wv_sb = wts.tile([128, D // 128, W], BF16, tag="wv")
            wo_sb = wts.tile([128, HLOC, D], BF16, tag="wo")
            nc.sync.dma_start(wq_sb[:], wq3.ap().rearrange("c p n -> p c n"))
            nc.sync.dma_start(wk_sb[:], wk3.ap().rearrange("c p n -> p c n"))
            nc.sync.dma_start(wv_sb[:], wv3.ap().rearrange("c p n -> p c n"))

            qT_sb = sb.tile([128, NPAIR], BF16, tag="qT")
            kT_sb = sb.tile([128, NPAIR], BF16, tag="kT")
            vrows = sb.tile([B, W], BF16, tag="vrows")

            with tc.tile_pool(name="psA", bufs=2, space="PSUM") as psA:
                rots = {}
                for wname, w_sb in (("q", wq_sb), ("k", wk_sb), ("v", wv_sb)):
                    prj = psA.tile([B, W], FP32, tag="prj")
                    for ci in range(D // 128):
                        nc.tensor.matmul(
                            prj[:],
                            xs[:, ci, :],
                            w_sb[:, ci, :],
                            start=(ci == 0),
                            stop=(ci == D // 128 - 1),
                        )
                    if wname == "v":
                        nc.vector.tensor_copy(vrows[:], prj[:])
                        continue
                    # RoPE in row layout: rot = prj*cos + swap(prj)*sin_signed
                    sw = sb.tile([B, W], FP32, tag="ropesw")
                    p3 = prj[:].rearrange("b (i two) -> b i two", two=2)
                    s3 = sw[:].rearrange("b (i two) -> b i two", two=2)
                    nc.vector.tensor_copy(s3[:, :, 0], p3[:, :, 1])
                    nc.vector.tensor_copy(s3[:, :, 1], p3[:, :, 0])
                    t1 = sb.tile([B, W], FP32, tag="ropet1")
                    t2 = sb.tile([B, W], FP32, tag="ropet2")
                    nc.vector.tensor_tensor(t1[:], prj[:], cos_sb[:], op=mult)
                    nc.vector.tensor_tensor(t2[:], sw[:], sin_sb[:], op=mult)
                    rot = sb.tile([B, W], FP32, tag=f"rot_{wname}")
                    nc.vector.tensor_tensor(rot[:], t1[:], t2[:], op=add)
                    rots[wname] = rot

                for h in range(HLOC):
                    for rot, dst in ((rots["q"], qT_sb), (rots["k"], kT_sb)):
                        tps = psA.tile([128, B], FP32, tag="tps")
                        nc.tensor.transpose(
                            tps[:],
                            rot[:, h * DH : (h + 1) * DH],
                            identity[:B, :B],
                        )
                        nc.vector.tensor_copy(
                            dst[:, h * B : (h + 1) * B], tps[:]
                        )

            # ---- phase B: attention over the cached prefix ----
            # Software-pipelined over pairs: pair p's V-matmuls are emitted
            # after pair p+1's score-matmuls so the PE never waits on the
            # exp round trip; K and V arrive in one merged 2MB DMA per pair.
            # per-pair softmax denominators accumulate for free via the exp's
            # accum_out; zero-padded tail columns each contribute exactly
            # exp(0) = 1, corrected with a compile-time constant below.
            accs = sb.tile([128, NPAIR], FP32, tag="accs")
            out_sb = sb.tile([B, D], FP32, tag="outsb")
            out_fin = sb.tile([B, D], FP32, tag="outfin")
            attn_sbs = []
            with (
                tc.tile_pool(name="psB", bufs=2, space="PSUM") as psB,
                tc.tile_pool(name="psacc", bufs=2, space="PSUM") as psacc,
                tc.tile_pool(name="psC", bufs=2, space="PSUM") as psC,
            ):
                attn_pss = []

                def emit_normalize(h):
                    # attn_sb = attn_ps * (1/sum); K=1 ones-matmul broadcasts
                    # the per-batch scalars across partitions
                    sums = psB.tile([1, B], FP32, tag="misc")
                    nc.tensor.matmul(
                        sums[:],
                        ones_colf[:],
                        accs[:, h * B : (h + 1) * B],
                        start=True,
                        stop=True,
                    )
                    stot_h = sb.tile([1, B], FP32, tag=f"stot{h}")
                    if r < 127:
                        nc.vector.tensor_scalar_add(
                            stot_h[:], sums[:], float(-(127 - r))
                        )
                    else:
                        nc.vector.tensor_copy(stot_h[:], sums[:])
                    inv_sb = sb.tile([1, B], FP32, tag=f"inv{h}")
                    nc.vector.reciprocal(inv_sb[:], stot_h[:])
                    binv = psB.tile([128, B], FP32, tag="misc")
                    nc.tensor.matmul(
                        binv[:], ones_row[:], inv_sb[:], start=True, stop=True
                    )
                    binv_sb = sb.tile([128, B], FP32, tag=f"binv{h}")
                    nc.vector.tensor_copy(binv_sb[:], binv[:])
                    attn_sb = sb.tile([128, B], BF16, tag=f"attnsb{h}")
                    nc.vector.tensor_tensor(
                        attn_sb[:], attn_pss[h][:], binv_sb[:], op=mult
                    )
                    attn_sbs.append(attn_sb)
                    # this head's slice of the output projection; head 0's
                    # runs mid-stream in PE slack, head 1's in the tail
                    for nt in range(D // 512):
                        ops = psC.tile([B, 512], FP32, tag="ops")
                        nc.tensor.matmul(
                            ops[:],
                            attn_sb[:],
                            wo_sb[:, h, nt * 512 : (nt + 1) * 512],
                            start=True,
                            stop=True,
                        )
                        dst = out_sb if h == 0 else out_fin
                        if h == 0:
                            nc.vector.tensor_copy(
                                dst[:, nt * 512 : (nt + 1) * 512], ops[:]
                            )
                        else:
                            nc.vector.tensor_tensor(
                                dst[:, nt * 512 : (nt + 1) * 512],
                                ops[:],
                                out_sb[:, nt * 512 : (nt + 1) * 512],
                                op=add,
                            )

                def emit_v(h, b, et, vt_v):
                    for ci in range(nch):
                        nc.tensor.matmul(
                            attn_pss[h][:, b : b + 1],
                            vt_v[:, ci * 128 : (ci + 1) * 128],
                            et[:, ci : ci + 1],
                            start=(ci == 0),
                            stop=(ci == nch - 1),
                        )
                    if b == B - 1:
                        emit_normalize(h)

                pending = None
                for h in range(HLOC):
                    attn_ps = psacc.tile([128, B], FP32, tag="attn")
                    attn_pss.append(attn_ps)
                    for b in range(B):
                        pcol = h * B + b
                        if pcol == 2:
                            # wo isn't needed until the first normalize; keep
                            # the startup window free for the q/k/v weights
                            nc.sync.dma_start(
                                wo_sb[:], wo3.ap().rearrange("h p n -> p h n")
                            )
                        kv_t = kpool.tile([128, 2, Tp], BF16, tag="kv")
                        nc.sync.dma_start(kv_t[:, 0, :], kv3.ap()[pcol, 0])
                        nc.sync.dma_start(kv_t[:, 1, :], kv3.ap()[pcol, 1])
                        kt_v = kv_t[:, 0, :]
                        vt_v = kv_t[:, 1, :]
                        # insert this step's (RoPE'd) k / v at t=start
                        nc.vector.tensor_copy(
                            kt_v[:, start : start + 1], kT_sb[:, pcol : pcol + 1]
                        )
                        nc.gpsimd.dma_start(
                            vt_v[r : r + 1, c_last * 128 : (c_last + 1) * 128],
                            vrows[b : b + 1, h * DH : (h + 1) * DH],
                        )
                        sc = psB.tile([128, nch], FP32, tag="sc")
                        for ci in range(nch):
                            nc.tensor.matmul(
                                sc[:, ci : ci + 1],
                                kt_v[:, ci * 128 : (ci + 1) * 128],
                                qT_sb[:, pcol : pcol + 1],
                                start=True,
                                stop=True,
                            )
                        if pending is not None:
                            emit_v(*pending)
                        et = etp.tile([128, nch], BF16, tag="et")
                        nc.scalar.activation(
                            et[:],
                            sc[:],
                            Exp,
                            scale=SCALE,
                            accum_out=accs[:, pcol : pcol + 1],
                        )
                        pending = (h, b, et, vt_v)
                emit_v(*pending)
            nc.sync.dma_start(outp.ap(), out_fin[:])

    _split_multi_waits(nc)
    return nc


_programs = {}


def _get_program(start):
    if start not in _programs:
        _programs[start] = _build_program(start)
    return _programs[start]


def _stage_inputs(inputs, key_cache, value_cache, freqs_cos, freqs_sin, Wq, Wk, Wv, Wo, start):
    nch = start // 128 + 1
    Tp = nch * 128
    r = start % 128

    f32 = np.float32
    bf16 = ml_dtypes.bfloat16
    x = np.asarray(inputs, f32).reshape(B, D)
    xT3 = np.ascontiguousarray(x.T, dtype=bf16).reshape(D // 128, 128, B)

    kc = np.asarray(key_cache, f32)[:, :Tp]  # [B, Tp, H, DH]
    vc = np.asarray(value_cache, f32)[:, :Tp]
    # One merged bf16 array per (head, batch) pair: [..., 0] = K^T [DH, Tp],
    # [..., 1] = V tiled [q, c*128+j] with t = c*128+q. bf16 halves the DMA
    # traffic; scores/attention still accumulate in fp32 on the PE.
    kv_all = np.empty((H, B, 2, 128, Tp), dtype=ml_dtypes.bfloat16)
    kv_all[:, :, 0] = kc.transpose(2, 0, 3, 1)
    kv_all[:, :, 1] = (
        vc.reshape(B, nch, 128, H, DH).transpose(3, 0, 2, 1, 4).reshape(H, B, 128, Tp)
    )
    if start + 1 < Tp:
        kv_all[:, :, 0, :, start + 1 :] = 0.0
        kv_all[:, :, 1, r + 1 :, (nch - 1) * 128 :] = 0.0

    fc = np.asarray(freqs_cos, f32).reshape(-1)[: DH // 2]
    fs = np.asarray(freqs_sin, f32).reshape(-1)[: DH // 2]
    cos128 = np.repeat(fc, 2)
    sin128 = np.repeat(fs, 2) * np.tile(np.array([-1.0, 1.0], f32), DH // 2)
    cos_row = np.ascontiguousarray(np.tile(cos128, HLOC)[None, :], dtype=f32)
    sin_row = np.ascontiguousarray(np.tile(sin128, HLOC)[None, :], dtype=f32)

    Wq = np.asarray(Wq, f32)
    Wk = np.asarray(Wk, f32)
    Wv = np.asarray(Wv, f32)
    Wo = np.asarray(Wo, f32)

    in_maps = []
    for c in range(NCORES):
        hs = slice(HLOC * c, HLOC * (c + 1))
        cols = slice(HLOC * c * DH, HLOC * (c + 1) * DH)
        in_maps.append(
            {
                "xT3": xT3,
                "wq3": np.ascontiguousarray(Wq[:, cols], dtype=bf16).reshape(D // 128, 128, HLOC * DH),
                "wk3": np.ascontiguousarray(Wk[:, cols], dtype=bf16).reshape(D // 128, 128, HLOC * DH),
                "wv3": np.ascontiguousarray(Wv[:, cols], dtype=bf16).reshape(D // 128, 128, HLOC * DH),
                "wo3": np.ascontiguousarray(Wo[cols, :], dtype=bf16).reshape(HLOC, 128, D),
                "cosr": cos_row,
                "sinr": sin_row,
                "kv3": kv_all[hs].reshape(NPAIR, 2, 128, Tp),
            }
        )
    return in_maps


def kernel(
    inputs,
    key_cache,
    value_cache,
    freqs_cos,
    freqs_sin,
    Wq,
    Wk,
    Wv,
    Wo,
    start_position,
    _trace=False,
    _tmpdir=None,
):
    global LAST_RESULT
    start = int(start_position)
    nc = _get_program(start)
    in_maps = _stage_inputs(
        inputs, key_cache, value_cache, freqs_cos, freqs_sin, Wq, Wk, Wv, Wo, start
    )
    res = run_bass_kernel_spmd(
        nc,
        in_maps,
        core_ids=list(range(NCORES)),
        trace=_trace,
        tmpdir=_tmpdir,
    )
    LAST_RESULT = res
    out = np.zeros((B, D), np.float32)
    for c in range(NCORES):
        out += res.results[c]["outp"]
    return out.reshape(B, 1, D)


# revision 15
# speedup vs baseline: 1.0860x; 1.0141x over previous
"""Trainium2 Bass kernel for single-token multi-head attention with KV cache
(B=16, S=1, D=2048, H=16, Dh=128, MAX_SEQ=4096), tensor-parallel over heads
across 8 NeuronCores (2 heads per core).

Per core:
  - q/k/v projections for the core's 2 heads (column-sliced Wq/Wk/Wv),
  - RoPE on q/k, KV-cache update at position `start_position`,
  - attention over the cached prefix (the memory-bound part: each core
    streams its 2-head slice of the K and V caches, 67 MB as bf16),
  - partial output projection with the row-slice of Wo.
The host sums the 8 partial [B, D] outputs (tensor-parallel unshard).

The K/V cache slices are staged host-side in bf16 (halves HBM traffic; all
matmuls still accumulate in fp32, softmax statistics are fp32) with layouts
chosen so every large DMA reads contiguous per-partition lines:
  kv[pair][0] = K^T  [Dh, Tp]               (scores keep K chunks stationary)
  kv[pair][1] = V as [q, c*128+j], t=c*128+q (V partitioned by t mod 128)
The per-pair GEMV attention runs as 128x128-stationary matmuls with N=1
moving vectors; exp runs on the scalar engine with accumulated row sums;
softmax denominators and normalization use ones-matmul partition reductions.
The pair loop is software-pipelined (pair p's V-matmuls are emitted after
pair p+1's score-matmuls) so the PE never stalls on the exp round trip.
"""

import math
import sys

sys.path.insert(0, "/opt/trn_rl_repo")

import numpy as np
import ml_dtypes

import concourse.bass as bass
import concourse.mybir as mybir
import concourse.tile as tile
from concourse.bass_utils import run_bass_kernel_spmd
from concourse.masks import make_identity

B, D, H, DH = 16, 2048, 16, 128
NCORES = 8
HLOC = H // NCORES  # heads per core
NPAIR = HLOC * B  # (head, batch) pairs per core
FP32 = mybir.dt.float32
BF16 = mybir.dt.bfloat16
SCALE = 1.0 / math.sqrt(DH)

LAST_RESULT = None  # BassKernelResults of the most recent run (for test harness)


def _split_multi_waits(nc):
    """walrus in this container accepts at most ONE sync wait per instruction
    (setupSyncWait: "Too many sync wait commands"). Tile's scheduler attaches
    several. Hoist all but the last wait of each instruction onto wait-only
    EventSemaphore instructions inserted right before it on the same engine —
    per-engine program order makes this semantically identical."""
    for f in nc.m.functions:
        for blk in f.blocks:
            insts = blk.instructions
            if not any(
                i.sync_info is not None and len(i.sync_info.on_wait) > 1
                for i in insts
            ):
                continue
            new = []
            for inst in insts:
                si = inst.sync_info
                if si is not None and len(si.on_wait) > 1:
                    waits = list(si.on_wait)
                    for j, w in enumerate(waits[:-1]):
                        es = mybir.InstEventSemaphore(
                            name=f"{inst.name}_hw{j}",
                            ins=[],
                            outs=[],
                            engine=inst.engine,
                        )
                        es.sync_info = mybir.SyncInfo(on_wait=[w], on_update=[])
                        new.append(es)
                    inst.sync_info = mybir.SyncInfo(
                        on_wait=[waits[-1]], on_update=list(si.on_update)
                    )
                new.append(inst)
            blk.instructions = new


def _build_program(start):
    """Bass program for one core (SPMD: all 8 cores run the same program on
    different data). `start` is the KV-cache write position; attention spans
    t in [0, start]."""
    nch = start // 128 + 1  # T-chunks of 128, padded
    Tp = nch * 128
    r = start % 128  # t=start lives at partition r of chunk nch-1
    c_last = nch - 1

    nc = bass.Bass(
        "TRN2", target_bir_lowering=False, debug=False, num_devices=NCORES
    )

    xT3 = nc.dram_tensor("xT3", [D // 128, 128, B], BF16, kind="ExternalInput")
    wq3 = nc.dram_tensor("wq3", [D // 128, 128, HLOC * DH], BF16, kind="ExternalInput")
    wk3 = nc.dram_tensor("wk3", [D // 128, 128, HLOC * DH], BF16, kind="ExternalInput")
    wv3 = nc.dram_tensor("wv3", [D // 128, 128, HLOC * DH], BF16, kind="ExternalInput")
    wo3 = nc.dram_tensor("wo3", [HLOC, 128, D], BF16, kind="ExternalInput")
    cosr = nc.dram_tensor("cosr", [1, HLOC * DH], FP32, kind="ExternalInput")
    sinr = nc.dram_tensor("sinr", [1, HLOC * DH], FP32, kind="ExternalInput")
    kv3 = nc.dram_tensor("kv3", [NPAIR, 2, 128, Tp], BF16, kind="ExternalInput")
    outp = nc.dram_tensor("outp", [B, D], FP32, kind="ExternalOutput")

    W = HLOC * DH  # 256: q/k/v row width for this core's heads
    Exp = mybir.ActivationFunctionType.Exp
    mult = mybir.AluOpType.mult
    add = mybir.AluOpType.add

    with tile.TileContext(nc) as tc:
        with (
            tc.tile_pool(name="consts", bufs=1) as consts,
            tc.tile_pool(name="sb", bufs=1) as sb,
            tc.tile_pool(name="wts", bufs=1) as wts,
            tc.tile_pool(name="kpool", bufs=6) as kpool,
            tc.tile_pool(name="etp", bufs=3) as etp,
        ):
            # ---- constants ----
            identity = consts.tile([128, 128], FP32, tag="identity")
            make_identity(nc, identity[:])
            ones_colf = consts.tile([128, 1], FP32, tag="ones_colf")
            nc.vector.memset(ones_colf[:], 1.0)
            ones_row = consts.tile([1, 128], FP32, tag="ones_row")
            nc.vector.memset(ones_row[:], 1.0)
            cos_sb = consts.tile([B, W], FP32, tag="cos")
            sin_sb = consts.tile([B, W], FP32, tag="sin")
            nc.gpsimd.dma_start(cos_sb[:], cosr.ap().to_broadcast((B, W)))
            nc.gpsimd.dma_start(sin_sb[:], sinr.ap().to_broadcast((B, W)))

            # ---- phase A: projections + RoPE + transposes ----
            xs = sb.tile([128, D // 128, B], BF16, tag="xs")
            nc.gpsimd.dma_start(xs[:], xT3.ap().rearrange("c p b -> p c b"))
            wq_sb = wts.tile([128, D // 128, W], BF16, tag="wq")
            wk_sb = wts.tile([128, D // 128, W], BF16, tag="wk")
            wv_sb = wts.tile([128, D // 128, W], BF16, tag="wv")
            wo_sb = wts.tile([128, HLOC, D], BF16, tag="wo")
            nc.sync.dma_start(wq_sb[:], wq3.ap().rearrange("c p n -> p c n"))
            nc.sync.dma_start(wk_sb[:], wk3.ap().rearrange("c p n -> p c n"))
            nc.sync.dma_start(wv_sb[:], wv3.ap().rearrange("c p n -> p c n"))

            qT_sb = sb.tile([128, NPAIR], BF16, tag="qT")
            kT_sb = sb.tile([128, NPAIR], BF16, tag="kT")
            vrows = sb.tile([B, W], BF16, tag="vrows")

            with tc.tile_pool(name="psA", bufs=2, space="PSUM") as psA:
                rots = {}
                for wname, w_sb in (("q", wq_sb), ("k", wk_sb), ("v", wv_sb)):
                    prj = psA.tile([B, W], FP32, tag="prj")
                    for ci in range(D // 128):
                        nc.tensor.matmul(
                            prj[:],
                            xs[:, ci, :],
                            w_sb[:, ci, :],
                            start=(ci == 0),
                            stop=(ci == D // 128 - 1),
                        )
                    if wname == "v":
                        nc.vector.tensor_copy(vrows[:], prj[:])
                        continue
                    # RoPE in row layout: rot = prj*cos + swap(prj)*sin_signed
                    sw = sb.tile([B, W], FP32, tag="ropesw")
                    p3 = prj[:].rearrange("b (i two) -> b i two", two=2)
                    s3 = sw[:].rearrange("b (i two) -> b i two", two=2)
                    nc.vector.tensor_copy(s3[:, :, 0], p3[:, :, 1])
                    nc.vector.tensor_copy(s3[:, :, 1], p3[:, :, 0])
                    t1 = sb.tile([B, W], FP32, tag="ropet1")
                    t2 = sb.tile([B, W], FP32, tag="ropet2")
                    nc.vector.tensor_tensor(t1[:], prj[:], cos_sb[:], op=mult)
                    nc.vector.tensor_tensor(t2[:], sw[:], sin_sb[:], op=mult)
                    rot = sb.tile([B, W], FP32, tag=f"rot_{wname}")
                    nc.vector.tensor_tensor(rot[:], t1[:], t2[:], op=add)
                    rots[wname] = rot

                for h in range(HLOC):
                    for rot, dst in ((rots["q"], qT_sb), (rots["k"], kT_sb)):
                        tps = psA.tile([128, B], FP32, tag="tps")
                        nc.tensor.transpose(
                            tps[:],
                            rot[:, h * DH : (h + 1) * DH],
                            identity[:B, :B],
                        )
                        nc.vector.tensor_copy(
                            dst[:, h * B : (h + 1) * B], tps[:]
                        )

            # ---- phase B: attention over the cached prefix ----
            # Software-pipelined over pairs: pair p's V-matmuls are emitted
            # after pair p+1's score-matmuls so the PE never waits on the
            # exp round trip; K and V arrive in one merged 2MB DMA per pair.
            # per-pair softmax denominators accumulate for free via the exp's
            # accum_out; zero-padded tail columns each contribute exactly
            # exp(0) = 1, corrected with a compile-time constant below.
            accs = sb.tile([128, NPAIR], FP32, tag="accs")
            out_sb = sb.tile([B, D], FP32, tag="outsb")
            out_fin = sb.tile([B, D], FP32, tag="outfin")
            attn_sbs = []
            with (
                tc.tile_pool(name="psB", bufs=2, space="PSUM") as psB,
                tc.tile_pool(name="psacc", bufs=2, space="PSUM") as psacc,
                tc.tile_pool(name="psC", bufs=2, space="PSUM") as psC,
            ):
                attn_pss = []

                def emit_normalize(h):
                    # attn_sb = attn_ps * (1/sum); K=1 ones-matmul broadcasts
                    # the per-batch scalars across partitions
                    sums = psB.tile([1, B], FP32, tag="misc")
                    nc.tensor.matmul(
                        sums[:],
                        ones_colf[:],
                        accs[:, h * B : (h + 1) * B],
                        start=True,
                        stop=True,
                    )
                    stot_h = sb.tile([1, B], FP32, tag=f"stot{h}")
                    if r < 127:
                        nc.vector.tensor_scalar_add(
                            stot_h[:], sums[:], float(-(127 - r))
                        )
                    else:
                        nc.vector.tensor_copy(stot_h[:], sums[:])
                    inv_sb = sb.tile([1, B], FP32, tag=f"inv{h}")
                    nc.vector.reciprocal(inv_sb[:], stot_h[:])
                    binv = psB.tile([128, B], FP32, tag="misc")
                    nc.tensor.matmul(
                        binv[:], ones_row[:], inv_sb[:], start=True, stop=True
                    )
                    binv_sb = sb.tile([128, B], FP32, tag=f"binv{h}")
                    nc.vector.tensor_copy(binv_sb[:], binv[:])
                    attn_sb = sb.tile([128, B], BF16, tag=f"attnsb{h}")
                    nc.vector.tensor_tensor(
                        attn_sb[:], attn_pss[h][:], binv_sb[:], op=mult
                    )
                    attn_sbs.append(attn_sb)
                    # this head's slice of the output projection; head 0's
                    # runs mid-stream in PE slack, head 1's in the tail
                    for nt in range(D // 512):
                        ops = psC.tile([B, 512], FP32, tag="ops")
                        nc.tensor.matmul(
                            ops[:],
                            attn_sb[:],
                            wo_sb[:, h, nt * 512 : (nt + 1) * 512],
                            start=True,
                            stop=True,
                        )
                        dst = out_sb if h == 0 else out_fin
                        if h == 0:
                            nc.vector.tensor_copy(
                                dst[:, nt * 512 : (nt + 1) * 512], ops[:]
                            )
                        else:
                            nc.vector.tensor_tensor(
                                dst[:, nt * 512 : (nt + 1) * 512],
                                ops[:],
                                out_sb[:, nt * 512 : (nt + 1) * 512],
                                op=add,
                            )

                def emit_v(h, b, et, vt_v):
                    for ci in range(nch):
                        nc.tensor.matmul(
                            attn_pss[h][:, b : b + 1],
                            vt_v[:, ci * 128 : (ci + 1) * 128],
                            et[:, ci : ci + 1],
                            start=(ci == 0),
                            stop=(ci == nch - 1),
                        )
                    if b == B - 1:
                        emit_normalize(h)

                pending = None
                for h in range(HLOC):
                    attn_ps = psacc.tile([128, B], FP32, tag="attn")
                    attn_pss.append(attn_ps)
                    for b in range(B):
                        pcol = h * B + b
                        if pcol == 2:
                            # wo isn't needed until the first normalize; keep
                            # the startup window free for the q/k/v weights
                            nc.sync.dma_start(
                                wo_sb[:], wo3.ap().rearrange("h p n -> p h n")
                            )
                        kv_t = kpool.tile([128, 2, Tp], BF16, tag="kv")
                        nc.sync.dma_start(kv_t[:, 0, :], kv3.ap()[pcol, 0])
                        nc.sync.dma_start(kv_t[:, 1, :], kv3.ap()[pcol, 1])
                        kt_v = kv_t[:, 0, :]
                        vt_v = kv_t[:, 1, :]
                        # insert this step's (RoPE'd) k / v at t=start
                        nc.vector.tensor_copy(
                            kt_v[:, start : start + 1], kT_sb[:, pcol : pcol + 1]
                        )
                        nc.gpsimd.dma_start(
                            vt_v[r : r + 1, c_last * 128 : (c_last + 1) * 128],
                            vrows[b : b + 1, h * DH : (h + 1) * DH],
                        )
                        sc = psB.tile([128, nch], FP32, tag="sc")
                        for ci in range(nch):
                            nc.tensor.matmul(
                                sc[:, ci : ci + 1],
                                kt_v[:, ci * 128 : (ci + 1) * 128],
                                qT_sb[:, pcol : pcol + 1],
                                start=True,
                                stop=True,
                            )
                        if pending is not None:
                            emit_v(*pending)
                        et = etp.tile([128, nch], BF16, tag="et")
                        nc.scalar.activation(
                            et[:],
                            sc[:],
                            Exp,
                            scale=SCALE,
                            accum_out=accs[:, pcol : pcol + 1],
                        )
                        pending = (h, b, et, vt_v)
                emit_v(*pending)
            nc.sync.dma_start(outp.ap(), out_fin[:])

    _split_multi_waits(nc)
    return nc


_programs = {}


def _get_program(start):
    if start not in _programs:
        _programs[start] = _build_program(start)
    return _programs[start]


def _stage_inputs(inputs, key_cache, value_cache, freqs_cos, freqs_sin, Wq, Wk, Wv, Wo, start):
    nch = start // 128 + 1
    Tp = nch * 128
    r = start % 128

    f32 = np.float32
    bf16 = ml_dtypes.bfloat16
    x = np.asarray(inputs, f32).reshape(B, D)
    xT3 = np.ascontiguousarray(x.T, dtype=bf16).reshape(D // 128, 128, B)

    kc = np.asarray(key_cache, f32)[:, :Tp]  # [B, Tp, H, DH]
    vc = np.asarray(value_cache, f32)[:, :Tp]
    # One merged bf16 array per (head, batch) pair: [..., 0] = K^T [DH, Tp],
    # [..., 1] = V tiled [q, c*128+j] with t = c*128+q. bf16 halves the DMA
    # traffic; scores/attention still accumulate in fp32 on the PE.
    kv_all = np.empty((H, B, 2, 128, Tp), dtype=ml_dtypes.bfloat16)
    kv_all[:, :, 0] = kc.transpose(2, 0, 3, 1)
    kv_all[:, :, 1] = (
        vc.reshape(B, nch, 128, H, DH).transpose(3, 0, 2, 1, 4).reshape(H, B, 128, Tp)
    )
    if start + 1 < Tp:
        kv_all[:, :, 0, :, start + 1 :] = 0.0
        kv_all[:, :, 1, r + 1 :, (nch - 1) * 128 :] = 0.0

    fc = np.asarray(freqs_cos, f32).reshape(-1)[: DH // 2]
    fs = np.asarray(freqs_sin, f32).reshape(-1)[: DH // 2]
    cos128 = np.repeat(fc, 2)
    sin128 = np.repeat(fs, 2) * np.tile(np.array([-1.0, 1.0], f32), DH // 2)
    cos_row = np.ascontiguousarray(np.tile(cos128, HLOC)[None, :], dtype=f32)
    sin_row = np.ascontiguousarray(np.tile(sin128, HLOC)[None, :], dtype=f32)

    Wq = np.asarray(Wq, f32)
    Wk = np.asarray(Wk, f32)
    Wv = np.asarray(Wv, f32)
    Wo = np.asarray(Wo, f32)

    in_maps = []
    for c in range(NCORES):
        hs = slice(HLOC * c, HLOC * (c + 1))
        cols = slice(HLOC * c * DH, HLOC * (c + 1) * DH)
        in_maps.append(
            {
                "xT3": xT3,
                "wq3": np.ascontiguousarray(Wq[:, cols], dtype=bf16).reshape(D // 128, 128, HLOC * DH),
                "wk3": np.ascontiguousarray(Wk[:, cols], dtype=bf16).reshape(D // 128, 128, HLOC * DH),
                "wv3": np.ascontiguousarray(Wv[:, cols], dtype=bf16).reshape(D // 128, 128, HLOC * DH),
                "wo3": np.ascontiguousarray(Wo[cols, :], dtype=bf16).reshape(HLOC, 128, D),
                "cosr": cos_row,
                "sinr": sin_row,
                "kv3": kv_all[hs].reshape(NPAIR, 2, 128, Tp),
            }
        )
    return in_maps


def kernel(
    inputs,
    key_cache,
    value_cache,
    freqs_cos,
    freqs_sin,
    Wq,
    Wk,
    Wv,
    Wo,
    start_position,
    _trace=False,
    _tmpdir=None,
):
    global LAST_RESULT
    start = int(start_position)
    nc = _get_program(start)
    in_maps = _stage_inputs(
        inputs, key_cache, value_cache, freqs_cos, freqs_sin, Wq, Wk, Wv, Wo, start
    )
    res = run_bass_kernel_spmd(
        nc,
        in_maps,
        core_ids=list(range(NCORES)),
        trace=_trace,
        tmpdir=_tmpdir,
    )
    LAST_RESULT = res
    out = np.zeros((B, D), np.float32)
    for c in range(NCORES):
        out += res.results[c]["outp"]
    return out.reshape(B, 1, D)
